# revision 4
# baseline (speedup 1.0000x reference)
"""Trainium2 Bass kernel for DistanceBasedAttention (L1-distance attention).

Contract: kernel(**inputs) takes FULL unsharded inputs (as produced by
setup_inputs()) and returns the FULL output [B, S, HID] float32.

Sharding: the 16 (batch, head) blocks are split 2-per-core across 8 cores
(core = b*4 + head_pair). Each core computes its two heads' attention output
and the partial out-projection (Wo rows of its heads); the host sums the four
per-batch partials and adds the effective bias (bv @ Wo + bo).

Math trick used on-device: with S = LAMBDA/sqrt(HD),
    D[j,i] = sum_d |q_id - k_jd| = 2*sum_d relu(q_id - k_jd) - Qsum[i] + Ksum[j]
The -Qsum[i] term is constant along the softmax axis (j) and cancels in
softmax, so it is never computed. The +Ksum[j] term is folded into the exp
activation's per-partition bias (ksb = -SCALE*Ksum[j] in j-partition layout,
computed by 16 tiny matmuls with hidT as stationary), so no rank-1 PSUM
corrections are needed.

relu tiles (one [128,512] tile per j-pair: partitions = 2 j's x 64 d, free =
512 i) are reduced over d by TensorEngine matmuls with 0/2 selector weights.
Two paths per 32-row column-group:
  - fp16 path: DVE tensor_scalar(add,max) produces fp16 tiles (4x packed
    mode); 16 matmuls of 512 rows each accumulate the group.
  - fp8 path: ScalarE Relu activation produces fp8e4 tiles; pairs of tiles
    are packed as [128,2,512] quads and consumed by 8 DoubleRow matmuls
    (0.5 PE cycles/row = 4x less PE time per tile than fp16).
The N8 knob sets how many of the 32 column-groups take the fp8 path,
balancing PE against ACT/DVE occupancy.
"""

import numpy as np

import concourse.bass as bass
import concourse.mybir as mybir
import concourse.tile as tile
from concourse.bass_utils import run_bass_kernel_spmd

F32 = mybir.dt.float32
F16 = mybir.dt.float16
F8 = mybir.dt.float8e4
Alu = mybir.AluOpType
Act = mybir.ActivationFunctionType
DR = mybir.MatmulPerfMode.DoubleRow

B, S, HID = 2, 512, 512
NH, HD = 8, 64
LAMBDA = 1.0
SCALE = float(LAMBDA / np.sqrt(HD))
N_CORES = 8

import os as _os
# number of the 32 (head,bank,group) column-groups on the fp8/ACT path
N8 = int(_os.environ.get("DBA_N8", "8"))
NGRP = 32
# residual fp16 A-tiles sent to ACT (out of ACT_D) inside fp16 groups
ACT_N = int(_os.environ.get("DBA_ACT_N", "0"))
ACT_D = 100
WORK_BUFS = int(_os.environ.get("DBA_WORK_BUFS", "12"))
PBANK_BUFS = int(_os.environ.get("DBA_PBANK_BUFS", "4"))


def _split_excess_waits(nc, max_waits=1):
    """walrus in this container accepts a single sync-wait per instruction;
    move excess waits onto same-engine NoOps inserted just before."""
    f = nc.m.functions[0]
    for bb in f.blocks:
        new_list = []
        changed = False
        for ins in bb.instructions:
            si = ins.sync_info
            if si is not None and si.on_wait is not None and len(si.on_wait) > max_waits:
                waits = list(si.on_wait)
                k = 0
                while len(waits) - k > max_waits:
                    chunk = waits[k : k + max_waits]
                    k += max_waits
                    nop = mybir.InstNoOp(name=f"{ins.name}-ws-{k}", ins=[], outs=[])
                    nop.engine = ins.engine
                    nop.sync_info = mybir.SyncInfo(on_wait=chunk, on_update=[])
                    new_list.append(nop)
                si.on_wait = waits[k:]
                changed = True
            new_list.append(ins)
        if changed:
            bb.instructions = new_list


def _build_program(repeat=0):
    nc = bass.Bass()
    hidt_d = nc.dram_tensor("hidt", [HID, S], F16, kind="ExternalInput")
    wq2_d = nc.dram_tensor("wq2", [HID, 256], F16, kind="ExternalInput")
    wk2n_d = nc.dram_tensor("wk2n", [HID, 128], F16, kind="ExternalInput")
    wv2_d = nc.dram_tensor("wv2", [HID, 128], F16, kind="ExternalInput")
    wo2_d = nc.dram_tensor("wo2", [128, HID], F16, kind="ExternalInput")
    wks2_d = nc.dram_tensor("wks2", [HID, 2], F16, kind="ExternalInput")
    bqcol_d = nc.dram_tensor("bqcol", [128, 2], F32, kind="ExternalInput")
    bkncol_d = nc.dram_tensor("bkncol", [128, 2], F32, kind="ExternalInput")
    bksT_d = nc.dram_tensor("bksT", [128, 2], F32, kind="ExternalInput")
    sel_d = nc.dram_tensor("sel", [128, 16, 32], F16, kind="ExternalInput")
    sel8_d = nc.dram_tensor("sel8", [128, 8, 2, 32], F8, kind="ExternalInput")
    ones64_d = nc.dram_tensor("ones64", [128, 64], F16, kind="ExternalInput")
    outp_d = nc.dram_tensor("outp", [S, HID], F32, kind="ExternalOutput")

    with tile.TileContext(nc) as tc:
        with (
            tc.tile_pool(name="consts", bufs=1) as consts,
            tc.tile_pool(name="work", bufs=WORK_BUFS) as work,
            tc.tile_pool(name="pbank", bufs=PBANK_BUFS, space="PSUM") as pbank,
            tc.tile_pool(name="paux", bufs=2, space="PSUM") as paux,
        ):
            if repeat:
                loop_cm = tc.For_i(
                    0, repeat, 1,
                    hint_engines=(
                        mybir.EngineType.DVE,
                        mybir.EngineType.Activation,
                        mybir.EngineType.PE,
                        mybir.EngineType.SP,
                    ),
                )
            else:
                import contextlib
                loop_cm = contextlib.nullcontext()
            with loop_cm:
                _emit_body(
                    nc, consts, work, pbank, paux,
                    hidt_d, wq2_d, wk2n_d, wv2_d, wo2_d, wks2_d,
                    bqcol_d, bkncol_d, bksT_d, sel_d, sel8_d, ones64_d, outp_d,
                )

    _split_excess_waits(nc)
    return nc


def _emit_body(
    nc, consts, work, pbank, paux,
    hidt_d, wq2_d, wk2n_d, wv2_d, wo2_d, wks2_d,
    bqcol_d, bkncol_d, bksT_d, sel_d, sel8_d, ones64_d, outp_d,
):
    # ---- DMAs: transposed fp16 hidden first (it gates the whole pipeline) ----
    hidT = consts.tile([128, 4, 512], F16, name="hidT")
    for kt in range(4):
        nc.sync.dma_start(hidT[:, kt, :], hidt_d[kt * 128 : (kt + 1) * 128, :])
    wq2 = consts.tile([128, 4, 256], F16, name="wq2")
    wk2n = consts.tile([128, 4, 128], F16, name="wk2n")
    wv2 = consts.tile([128, 4, 128], F16, name="wv2")
    for kt in range(4):
        nc.sync.dma_start(wq2[:, kt, :], wq2_d[kt * 128 : (kt + 1) * 128, :])
        nc.sync.dma_start(wk2n[:, kt, :], wk2n_d[kt * 128 : (kt + 1) * 128, :])
    sel = consts.tile([128, 16, 32], F16, name="sel")
    nc.sync.dma_start(sel[:], sel_d[:])
    sel8 = consts.tile([128, 8, 2, 32], F8, name="sel8")
    nc.sync.dma_start(sel8[:], sel8_d[:])
    bqcol = consts.tile([128, 2], F32, name="bqcol")
    nc.sync.dma_start(bqcol[:], bqcol_d[:])
    bkncol = consts.tile([128, 2], F32, name="bkncol")
    nc.sync.dma_start(bkncol[:], bkncol_d[:])
    bksT = consts.tile([128, 2], F32, name="bksT")
    nc.sync.dma_start(bksT[:], bksT_d[:])
    wks2 = consts.tile([128, 4, 2], F16, name="wks2")
    for kt in range(4):
        nc.sync.dma_start(wks2[:, kt, :], wks2_d[kt * 128 : (kt + 1) * 128, :])
    for kt in range(4):
        nc.sync.dma_start(wv2[:, kt, :], wv2_d[kt * 128 : (kt + 1) * 128, :])
    ones64 = consts.tile([128, 64], F16, name="ones64")
    nc.sync.dma_start(ones64[:], ones64_d[:])
    wo2 = consts.tile([128, 512], F16, name="wo2")
    nc.sync.dma_start(wo2[:], wo2_d[:])

    def hidT_par(kt, par):
        return hidT[:, kt].rearrange("p (j two) -> p two j", two=2)[:, par, :]

    # ---- Q^T / -K^T per head (head 0 copies on DVE: ACT still idle-ish) ----
    qt2, ktp = [], []

    def emit_qkt(h):
        q_ps = pbank.tile([128, 512], F32, name="q_ps", tag="bank")
        for kt in range(4):
            nc.tensor.matmul(
                q_ps[:],
                wq2[:, kt, 128 * h : 128 * h + 128],
                hidT[:, kt, :],
                start=(kt == 0), stop=(kt == 3),
            )
        q_sb = consts.tile([128, 512], F16, name=f"qt2_{h}")
        if h == 0:
            nc.vector.tensor_scalar(
                q_sb[:], q_ps[:], bqcol[:, h : h + 1], None, Alu.add
            )
        else:
            nc.scalar.activation(
                q_sb[:], q_ps[:], Act.Identity, bias=bqcol[:, h : h + 1], scale=1.0
            )
        qt2.append(q_sb)

        k_ps = pbank.tile([128, 256], F32, name="k_ps", tag="bank")
        for par in range(2):
            for kt in range(4):
                nc.tensor.matmul(
                    k_ps[64 * par : 64 * par + 64, :],
                    wk2n[:, kt, 64 * h : 64 * h + 64],
                    hidT_par(kt, par),
                    start=(kt == 0), stop=(kt == 3),
                    tile_position=(0, 64 * par),
                )
        k_sb = consts.tile([128, 256], F32, name=f"ktp_{h}")
        # ktp copy on ACT for BOTH heads: runs concurrently with the DVE qt2
        # copy in the prologue, so neither engine's first A-tile waits on both.
        nc.scalar.activation(
            k_sb[:], k_ps[:], Act.Identity, bias=bkncol[:, h : h + 1], scale=1.0
        )
        ktp.append(k_sb)

    emit_qkt(0)

    # ---- Ksum in j-partition layout: ksb[j, (bk,h)] = -SCALE*(Ksum[j]+bksum)
    # (wks2 is pre-scaled by -SCALE host-side; bksT carries -SCALE*bksum).
    # 16 tiny matmuls: stationary hidT chunk [128hid,128j], moving wks2 [128hid,2].
    # One PSUM tile per bk chunk: PSUM zero-regions are 2KB, so concurrent
    # accumulation groups must not share a region.
    ksb = consts.tile([128, 4, 2], F32, name="ksb")
    for bk in range(4):
        ks_ps = paux.tile([128, 2], F32, name="ks_ps", tag="aux")
        for kt in range(4):
            nc.tensor.matmul(
                ks_ps[:],
                hidT[:, kt, 128 * bk : 128 * bk + 128],
                wks2[:, kt, :],
                start=(kt == 0), stop=(kt == 3),
            )
        for h in range(2):
            nc.vector.tensor_scalar(
                ksb[:, bk, h : h + 1], ks_ps[:, h : h + 1],
                bksT[:, h : h + 1], None, Alu.add,
            )

    # ---- distance banks ----
    et_sb = [consts.tile([128, 4, 512], F16, name=f"et_{h}") for h in range(2)]
    tile_state = {"ctr": 0, "grp": 0}
    pending_exp = []

    def flush_exp():
        while pending_exp:
            ph, pbk, pdt = pending_exp.pop(0)
            nc.scalar.activation(
                et_sb[ph][:, pbk, :], pdt[:], Act.Exp,
                bias=ksb[:, pbk, ph : ph + 1], scale=-SCALE,
            )

    def emit_bank(h, bk):
        dt_ps = pbank.tile([128, 512], F32, name="dt_ps", tag="bank")
        # which of this bank's 4 column-groups take the fp8/ACT path; fp8
        # groups sit at the LOW column positions (DoubleRow's dst-partition
        # range is restricted to the 64-wide perf-mode array).
        bi = tile_state["grp"]
        tile_state["grp"] = bi + 1
        n8_here = (N8 // 8) + (1 if bi < (N8 % 8) else 0)
        paths = [g < n8_here for g in range(4)]
        for t in range(16):
            if t == 5:
                flush_exp()
            for g in range(4):
                if paths[g]:
                    if t % 2:
                        continue
                    tp = t // 2
                    jp0 = 64 * bk + 16 * g + 2 * tp
                    a8 = work.tile([128, 2, 512], F8, name="a8", tag="a")
                    for i in range(2):
                        nc.scalar.activation(
                            a8[:, i, :], qt2[h][:], Act.Relu,
                            bias=ktp[h][:, jp0 + i : jp0 + i + 1], scale=1.0,
                        )
                    nc.tensor.matmul(
                        dt_ps[32 * g : 32 * g + 32, :],
                        sel8[:, tp, :, :], a8[:],
                        start=(tp == 0), stop=(tp == 7),
                        perf_mode=DR,
                        tile_position=(0, 32 * g),
                    )
                else:
                    jp = 64 * bk + 16 * g + t
                    a = work.tile([128, 512], F16, name="a", tag="a")
                    c = tile_state["ctr"]
                    use_act = ((c * ACT_N) // ACT_D) != (((c + 1) * ACT_N) // ACT_D)
                    tile_state["ctr"] = c + 1
                    if use_act:
                        nc.scalar.activation(
                            a[:], qt2[h][:], Act.Relu,
                            bias=ktp[h][:, jp : jp + 1], scale=1.0,
                        )
                    else:
                        nc.vector.tensor_scalar(
                            a[:], qt2[h][:], ktp[h][:, jp : jp + 1], 0.0,
                            Alu.add, Alu.max,
                        )
                    nc.tensor.matmul(
                        dt_ps[32 * g : 32 * g + 32, :],
                        sel[:, t, :], a[:],
                        start=(t == 0), stop=(t == 15),
                        tile_position=(0, 32 * g),
                    )
        pending_exp.append((h, bk, dt_ps))

    emit_bank(0, 0)
    emit_bank(0, 1)
    emit_qkt(1)
    emit_bank(0, 2)
    emit_bank(0, 3)
    emit_bank(1, 0)
    v_sb = consts.tile([128, 4, 128], F16, name="v_sb")
    for jt in range(4):
        v_ps = paux.tile([128, 128], F32, name="v_ps", tag="aux")
        for kt in range(4):
            nc.tensor.matmul(
                v_ps[:], hidT[:, kt, jt * 128 : (jt + 1) * 128],
                wv2[:, kt, :],
                start=(kt == 0), stop=(kt == 3),
            )
        nc.vector.tensor_copy(v_sb[:, jt, :], v_ps[:])
    for bk in range(1, 4):
        emit_bank(1, bk)
    flush_exp()

    # ---- softmax normalize + AV per head ----
    normT = consts.tile([128, 512], F16, name="normT")
    for h in range(2):
        cs_ps = paux.tile([64, 512], F32, name="cs_ps", tag="aux")
        for jt in range(4):
            nc.tensor.matmul(
                cs_ps[:], ones64[:], et_sb[h][:, jt, :],
                start=(jt == 0), stop=(jt == 3),
            )
        recip = consts.tile([64, 512], F32, name=f"recip{h}")
        nc.vector.reciprocal(recip[:], cs_ps[:])
        av_ps = paux.tile([64, 512], F32, name="av_ps", tag="aux")
        for jt in range(4):
            nc.tensor.matmul(
                av_ps[:], v_sb[:, jt, 64 * h : 64 * h + 64],
                et_sb[h][:, jt, :],
                start=(jt == 0), stop=(jt == 3),
            )
        nc.vector.tensor_mul(normT[64 * h : 64 * h + 64, :], av_ps[:], recip[:])

    # ---- out-projection partial + store ----
    for st in range(4):
        f_ps = pbank.tile([128, 512], F32, name="f_ps", tag="bank")
        nc.tensor.matmul(
            f_ps[:], normT[:, st * 128 : (st + 1) * 128], wo2[:],
            start=True, stop=True,
        )
        o_sb = work.tile([128, 512], F32, name="o_sb", tag="o")
        if st % 2 == 0:
            nc.vector.tensor_copy(o_sb[:], f_ps[:])
        else:
            nc.scalar.copy(o_sb[:], f_ps[:])
        nc.sync.dma_start(outp_d[st * 128 : (st + 1) * 128, :], o_sb[:])


_NC = None


def _get_nc():
    global _NC
    if _NC is None:
        _NC = _build_program()
    return _NC


def _host_constants():
    import ml_dtypes
    sel = np.zeros((128, 16, 32), np.float16)
    for t in range(16):
        for p in range(128):
            sel[p, t, 2 * t + p // 64] = 2.0
    sel8 = np.zeros((128, 8, 2, 32), np.float32)
    for tp in range(8):
        for i in range(2):
            for p in range(128):
                sel8[p, tp, i, 4 * tp + 2 * i + p // 64] = 2.0
    sel8 = sel8.astype(ml_dtypes.float8_e4m3)
    ones64 = np.ones((128, 64), np.float16)
    return sel, sel8, ones64


def kernel(hidden_states, Wq, bq, Wk, bk, Wv, bv, Wo, bo):
    hidden_states = np.asarray(hidden_states, np.float32)
    Wq, bq = np.asarray(Wq, np.float32), np.asarray(bq, np.float32)
    Wk, bk = np.asarray(Wk, np.float32), np.asarray(bk, np.float32)
    Wv, bv = np.asarray(Wv, np.float32), np.asarray(bv, np.float32)
    Wo, bo = np.asarray(Wo, np.float32), np.asarray(bo, np.float32)

    sel, sel8, ones64 = _host_constants()
    in_maps = []
    for core in range(N_CORES):
        b = core // 4
        hp = core % 4
        cols = slice(hp * 128, hp * 128 + 128)
        wk_sl = Wk[:, cols]
        bq_sl, bk_sl = bq[cols.start : cols.stop], bk[cols.start : cols.stop]
        bqcol = np.stack(
            [np.tile(bq_sl[lh * 64 : lh * 64 + 64], 2) for lh in range(2)], axis=1
        ).astype(np.float32)
        bkncol = np.stack(
            [np.tile(-bk_sl[lh * 64 : lh * 64 + 64], 2) for lh in range(2)], axis=1
        ).astype(np.float32)
        bksT = np.tile(
            np.array(
                [[-SCALE * bk_sl[0:64].sum(), -SCALE * bk_sl[64:128].sum()]],
                np.float32,
            ),
            (128, 1),
        )
        in_maps.append(
            {
                "hidt": np.ascontiguousarray(hidden_states[b].T).astype(np.float16),
                "wq2": np.concatenate(
                    [
                        np.concatenate([Wq[:, cols][:, l * 64 : l * 64 + 64]] * 2, axis=1)
                        for l in range(2)
                    ],
                    axis=1,
                ).astype(np.float16),
                "wk2n": np.ascontiguousarray(-wk_sl).astype(np.float16),
                "wv2": np.ascontiguousarray(Wv[:, cols]).astype(np.float16),
                "wo2": np.ascontiguousarray(Wo[cols, :]).astype(np.float16),
                "wks2": (-SCALE * wk_sl.reshape(HID, 2, 64).sum(-1)).astype(
                    np.float16
                ),
                "bqcol": bqcol,
                "bkncol": bkncol,
                "bksT": bksT,
                "sel": sel,
                "sel8": sel8,
                "ones64": ones64,
            }
        )

    nc = _get_nc()
    res = run_bass_kernel_spmd(nc, in_maps, core_ids=list(range(N_CORES)))
    parts = [r["outp"] for r in res.results]
    bo_eff = bv @ Wo + bo
    out = np.stack(
        [
            parts[0] + parts[1] + parts[2] + parts[3],
            parts[4] + parts[5] + parts[6] + parts[7],
        ],
        axis=0,
    )
    return (out + bo_eff[None, None, :]).astype(np.float32)


if __name__ == "__main__":
    rng = np.random.default_rng(0)
    w = 0.02
    inputs = {
        "hidden_states": rng.standard_normal((B, S, HID)).astype(np.float32),
        "Wq": (rng.standard_normal((HID, HID)) * w).astype(np.float32),
        "bq": np.zeros(HID, np.float32),
        "Wk": (rng.standard_normal((HID, HID)) * w).astype(np.float32),
        "bk": np.zeros(HID, np.float32),
        "Wv": (rng.standard_normal((HID, HID)) * w).astype(np.float32),
        "bv": np.zeros(HID, np.float32),
        "Wo": (rng.standard_normal((HID, HID)) * w).astype(np.float32),
        "bo": np.zeros(HID, np.float32),
    }
    out = kernel(**inputs)
    print("out shape:", out.shape, "finite:", np.isfinite(out).all())
